# revision 24
# baseline (speedup 1.0000x reference)
"""Trainium2 Bass kernel for a fused MultiHeadAttention block.

Reference computation (B=4, S=1024, D=1024, H=16, DK=DV=64):
    qh = einsum('bqd,hdk->bhqk', q, wq); kh, vh likewise
    attn = softmax(mask_fill(qh/sqrt(DK) @ kh^T))
    out  = LayerNorm(concat_heads(attn @ vh) @ fc_w.T + q) * ln_g + ln_b

Sharding: 8 shards = (batch b, seq half).  Each core owns 512 query rows of
one batch; K/V projections for that batch are computed redundantly by the
core pair.  Zero collectives.

v2 strategy (vs the fp32 baseline):
  - ALL layout work happens on the host: q/k/v/mask arrive pre-transposed
    (contraction dim on partitions), weights pre-packed per head pair, and
    everything cast to bf16.  No on-chip PE transposes, no psum evacuation
    copies for layout, half the DMA bytes.
  - every matmul runs in bf16 (1 cyc/row, same as fp32r, but transposes
    and elementwise work get 2x/4x DVE modes and half the SBUF footprint).
  - scores are computed TRANSPOSED [k_part, q_free]; softmax needs no max
    pass (|scores| <~ 6 sigma, bf16 exp cannot overflow), masking is
    p = exp(scores) * mask, row-sums come from an appended ones-column in
    vh, applied during the PV-psum evacuation.
  - loop order: vh proj; then per head-pair {kh/qh proj, scores, exp, PV}
    so the PE works on pair p+1's projections while Act exps pair p.
  - fc + residual + LayerNorm per 128-row tile at the end.
"""

import os
import sys

import numpy as np

for _p in ("/opt/trn_rl_repo",):
    if _p not in sys.path and os.path.isdir(_p):
        sys.path.insert(0, _p)

from contextlib import ExitStack

import ml_dtypes

import concourse.bass as bass
import concourse.tile as tile
from concourse import bacc, mybir
from concourse.bass_utils import run_bass_kernel_spmd

F32 = mybir.dt.float32
BF16 = mybir.dt.bfloat16
AF = mybir.ActivationFunctionType
NPBF16 = ml_dtypes.bfloat16

B, S, D = 4, 1024, 1024
H, DK, DV = 16, 64, 64
SQ = S // 2          # query rows per core
P = 128
NDC = D // P         # 8 contraction chunks over D
NKC = S // P         # 8 key chunks
NQT = SQ // P        # 4 query subtiles
NPAIR = H // 2       # 8 head pairs
TEMP_INV = 1.0 / 8.0  # 1/sqrt(DK), folded into qT on the host
LN_EPS = 1e-6
N_CORES = 8
VW = DV + 1          # vh columns incl. the ones-column for row sums
VPAD = 65            # vh stride
NKCH = NKC // 2      # vh is split in two tiles of 4 key-chunks each


def build_program(reps: int = 1):
    nc = bacc.Bacc("TRN2", target_bir_lowering=False, debug=False)

    qT_d = nc.dram_tensor("qT_sh", [P, NDC, SQ], BF16, kind="ExternalInput")
    kT_d = nc.dram_tensor("kT_full", [P, NDC, S], BF16, kind="ExternalInput")
    vT_d = nc.dram_tensor("vT_full", [P, NDC, S], BF16, kind="ExternalInput")
    mT_d = nc.dram_tensor("mT_sh", [P, NKC, SQ], BF16, kind="ExternalInput")
    wq_d = nc.dram_tensor("wq_p", [P, NDC, H * DK], BF16, kind="ExternalInput")
    wk_d = nc.dram_tensor("wk_p", [P, NDC, H * DK], BF16, kind="ExternalInput")
    wv_d = nc.dram_tensor("wv_p", [P, NDC, H * DV], BF16, kind="ExternalInput")
    fcT_d = nc.dram_tensor("fcT_p", [P, NDC, D], BF16, kind="ExternalInput")
    qr_d = nc.dram_tensor("qr_sh", [P, NQT, D], BF16, kind="ExternalInput")
    g_d = nc.dram_tensor("ln_g", [D], BF16, kind="ExternalInput")
    b_d = nc.dram_tensor("ln_b", [D], BF16, kind="ExternalInput")
    o_d = nc.dram_tensor("out_sh", [SQ, D], F32, kind="ExternalOutput")

    with tile.TileContext(nc) as tc, ExitStack() as ctx:
        singles = ctx.enter_context(tc.tile_pool(name="singles", bufs=1))
        ins = ctx.enter_context(tc.tile_pool(name="ins", bufs=1))
        mid = ctx.enter_context(tc.tile_pool(name="mid", bufs=1))
        work = ctx.enter_context(tc.tile_pool(name="work", bufs=2))
        vha_pool = ctx.enter_context(tc.tile_pool(name="vha", bufs=2))
        pwork = ctx.enter_context(tc.tile_pool(name="pwork", bufs=4))
        ps_proj = ctx.enter_context(
            tc.tile_pool(name="ps_proj", bufs=2, space="PSUM"))
        ps_sc = ctx.enter_context(
            tc.tile_pool(name="ps_sc", bufs=2, space="PSUM"))
        ps_hd = ctx.enter_context(
            tc.tile_pool(name="ps_hd", bufs=2, space="PSUM"))

        zero1 = singles.tile([P, 1], F32, tag="zero1")
        nc.vector.memset(zero1, 0.0)
        eps1 = singles.tile([P, 1], F32, tag="eps1")
        nc.vector.memset(eps1, LN_EPS)

        def _one_rep():
            # -- input DMAs, split over two queues, in consumption order --
            wv_sb = ins.tile([P, NDC, H * DV], BF16, tag="wv")
            vT_sb = ins.tile([P, NDC, S], BF16, tag="vT")
            wk_sb = ins.tile([P, NDC, H * DK], BF16, tag="wk")
            wq_sb = ins.tile([P, NDC, H * DK], BF16, tag="wq")
            kT_sb = ins.tile([P, NDC, S], BF16, tag="kT")
            qT_sb = ins.tile([P, NDC, SQ], BF16, tag="qT")
            mT_sb = ins.tile([P, NKC, SQ], BF16, tag="mT")
            fcT_sb = ins.tile([P, NDC, D], BF16, tag="fcT")
            qr_sb = ins.tile([P, NQT, D], BF16, tag="qr")
            gb = ins.tile([P, 2, D], BF16, tag="gb")

            nc.sync.dma_start(out=wv_sb, in_=wv_d[:])
            nc.sync.dma_start(out=vT_sb, in_=vT_d[:])
            nc.sync.dma_start(out=wk_sb, in_=wk_d[:])
            nc.sync.dma_start(out=wq_sb, in_=wq_d[:])
            nc.sync.dma_start(out=kT_sb, in_=kT_d[:])
            nc.sync.dma_start(out=qT_sb, in_=qT_d[:])
            nc.gpsimd.dma_start(out=mT_sb, in_=mT_d[:])
            nc.gpsimd.dma_start(out=fcT_sb, in_=fcT_d[:])
            nc.gpsimd.dma_start(out=qr_sb, in_=qr_d[:])
            nc.gpsimd.dma_start(
                out=gb[:, 0, :], in_=g_d.ap().unsqueeze(0).to_broadcast([P, D]))
            nc.gpsimd.dma_start(
                out=gb[:, 1, :], in_=b_d.ap().unsqueeze(0).to_broadcast([P, D]))

            # -- vh projection: vh[key_p, kc, h, 0:64] = vh, col 64 = 1 --
            # split in two tiles; the first is double-buffered so the next
            # rep's vh evacuations can start while this rep still reads it.
            vhA = vha_pool.tile([P, NKCH, H, VPAD], BF16, tag="vhA")
            vhB = mid.tile([P, NKCH, H, VPAD], BF16, tag="vhB")

            def vh_tile(kc):
                t = vhA if kc < NKCH else vhB
                return t[:, kc % NKCH]

            nc.vector.memset(vhA[:, :, :, DV:DV + 1], 1.0)
            nc.vector.memset(vhB[:, :, :, DV:DV + 1], 1.0)
            for kc in range(NKC):
                for hf in range(2):
                    vps = ps_proj.tile([P, 512], F32, tag="proj")
                    for dc in range(NDC):
                        nc.tensor.matmul(
                            vps,
                            lhsT=vT_sb[:, dc, kc * P:(kc + 1) * P],
                            rhs=wv_sb[:, dc, hf * 512:(hf + 1) * 512],
                            start=(dc == 0), stop=(dc == NDC - 1))
                    nc.scalar.copy(
                        out=vh_tile(kc)[:, hf * 8:(hf + 1) * 8, 0:DV],
                        in_=vps.rearrange("p (h v) -> p h v", v=DV))

            # -- per head-pair: kh/qh proj, then attention for both heads.
            # proj matmuls of pair p+1 are interleaved into pair p's
            # attention groups as PE filler, and each PV group trails its
            # scores by 2 slots, so the PE never waits on the exp chain. --
            khT = mid.tile([P, NPAIR, S], BF16, tag="khT")
            qhT = mid.tile([P, NPAIR, SQ], BF16, tag="qhT")
            concatT = mid.tile([P, NPAIR, SQ], BF16, tag="concatT")

            def proj_gen(pair):
                """Yields after each of the 24 proj matmuls of `pair`."""
                cols = slice(pair * P, (pair + 1) * P)
                qhps = ps_proj.tile([P, 512], F32, tag="proj")
                for dc in range(NDC):
                    nc.tensor.matmul(
                        qhps, lhsT=wq_sb[:, dc, cols], rhs=qT_sb[:, dc, :],
                        start=(dc == 0), stop=(dc == NDC - 1))
                    if dc == NDC - 1:
                        nc.scalar.copy(out=qhT[:, pair, :], in_=qhps)
                    yield
                for hf in range(2):
                    khps = ps_proj.tile([P, 512], F32, tag="proj")
                    for dc in range(NDC):
                        nc.tensor.matmul(
                            khps, lhsT=wk_sb[:, dc, cols],
                            rhs=kT_sb[:, dc, hf * 512:(hf + 1) * 512],
                            start=(dc == 0), stop=(dc == NDC - 1))
                        if dc == NDC - 1:
                            nc.scalar.copy(
                                out=khT[:, pair, hf * 512:(hf + 1) * 512],
                                in_=khps)
                        yield

            def drain(gen, n):
                if gen is None:
                    return
                for _ in range(n):
                    next(gen, None)

            hd_map = {}

            def emit_pv(ent):
                h, kc2, p_sb = ent
                if h not in hd_map:
                    hd_map[h] = ps_hd.tile([P, SQ], F32, tag="hd", name="hd")
                hd = hd_map[h]
                for j in range(2):
                    kc = 2 * kc2 + j
                    nc.tensor.matmul(
                        hd[0:VW, :], lhsT=vh_tile(kc)[:, h, 0:VW],
                        rhs=p_sb[:, j, :],
                        start=(kc == 0), stop=(kc == NKC - 1))
                if kc2 == NKC // 2 - 1:
                    # normalize rows 0:64 by the rowsum in row 64
                    hl = h % 2
                    recip = work.tile([1, SQ], F32, tag="recip")
                    nc.vector.reciprocal(out=recip, in_=hd[DV:DV + 1, :])
                    recip_bc = work.tile([DV, SQ], F32, tag="recip_bc")
                    nc.gpsimd.partition_broadcast(recip_bc, recip)
                    nc.vector.tensor_mul(
                        concatT[hl * DV:(hl + 1) * DV, h // 2, :],
                        hd[0:DV, :], recip_bc)
                    del hd_map[h]

            drain(proj_gen(0), 24)
            pending = []
            for pair in range(NPAIR):
                filler = proj_gen(pair + 1) if pair + 1 < NPAIR else None
                for hl in range(2):
                    h = 2 * pair + hl
                    hrows = slice(hl * DK, (hl + 1) * DK)
                    for kc2 in range(NKC // 2):
                        sc = ps_sc.tile([P, 2, SQ], F32, tag="sc")
                        for j in range(2):
                            kc = 2 * kc2 + j
                            nc.tensor.matmul(
                                sc[:, j, :],
                                lhsT=khT[hrows, pair, kc * P:(kc + 1) * P],
                                rhs=qhT[hrows, pair, :],
                                start=True, stop=True)
                        p_sb = pwork.tile([P, 2, SQ], BF16, tag="p_sb")
                        nc.scalar.activation(
                            out=p_sb, in_=sc, func=AF.Exp, bias=zero1)
                        nc.vector.tensor_mul(
                            p_sb, p_sb, mT_sb[:, 2 * kc2:2 * kc2 + 2, :])
                        drain(filler, 3)
                        pending.append((h, kc2, p_sb))
                        if len(pending) > 2:
                            emit_pv(pending.pop(0))
            for ent in pending:
                emit_pv(ent)

            # -- fc + residual + LayerNorm per 128-row tile --
            for st in range(NQT):
                o_sb = work.tile([P, D], F32, tag="o_sb")
                for hf in range(2):
                    fps = ps_proj.tile([P, 512], F32, tag="proj")
                    for ic in range(NDC):
                        nc.tensor.matmul(
                            fps,
                            lhsT=concatT[:, ic, st * P:(st + 1) * P],
                            rhs=fcT_sb[:, ic, hf * 512:(hf + 1) * 512],
                            start=(ic == 0), stop=(ic == NDC - 1))
                    nc.vector.tensor_add(
                        o_sb[:, hf * 512:(hf + 1) * 512], fps,
                        qr_sb[:, st, hf * 512:(hf + 1) * 512])
                stats = work.tile([P, 2, 6], F32, tag="stats")
                for sg in range(2):
                    nc.vector.bn_stats(
                        out=stats[:, sg, :],
                        in_=o_sb[:, sg * 512:(sg + 1) * 512])
                mv = work.tile([P, 2], F32, tag="mv")
                nc.vector.bn_aggr(out=mv, in_=stats)
                std = work.tile([P, 1], F32, tag="std")
                nc.scalar.activation(
                    out=std, in_=mv[:, 1:2], func=AF.Sqrt, bias=eps1)
                rstd = work.tile([P, 1], F32, tag="rstd")
                nc.vector.reciprocal(out=rstd, in_=std)
                nc.vector.tensor_scalar(
                    out=o_sb, in0=o_sb, scalar1=mv[:, 0:1], scalar2=rstd,
                    op0=mybir.AluOpType.subtract, op1=mybir.AluOpType.mult)
                nc.gpsimd.tensor_mul(o_sb, o_sb, gb[:, 0, :])
                nc.gpsimd.tensor_add(o_sb, o_sb, gb[:, 1, :])
                nc.gpsimd.dma_start(
                    out=o_d[st * P:(st + 1) * P, :], in_=o_sb)

        for _rep in range(reps):
            _one_rep()

    nc.compile()
    return nc


_CACHE = {}


def _get_program():
    if "nc" not in _CACHE:
        _CACHE["nc"] = build_program()
    return _CACHE["nc"]


def _to_pds(x_t, nfree):
    """[d, n] (d-major) -> [128, d//128, n] partition-dim-split layout."""
    d = x_t.shape[0]
    return np.ascontiguousarray(
        x_t.reshape(d // P, P, nfree).transpose(1, 0, 2))


def make_in_maps(q, k, v, mask, wq, wk, wv, fc_w, ln_g, ln_b):
    q = np.asarray(q, dtype=np.float32)
    k = np.asarray(k, dtype=np.float32)
    v = np.asarray(v, dtype=np.float32)
    mask = np.asarray(mask, dtype=np.int32)
    # weights, packed [p, dc, h*64+j] and cast to bf16 (shared by all cores)
    wq_p = _to_pds(np.asarray(wq).transpose(1, 0, 2).reshape(D, H * DK)
                   .astype(NPBF16), H * DK)
    wk_p = _to_pds(np.asarray(wk).transpose(1, 0, 2).reshape(D, H * DK)
                   .astype(NPBF16), H * DK)
    wv_p = _to_pds(np.asarray(wv).transpose(1, 0, 2).reshape(D, H * DV)
                   .astype(NPBF16), H * DV)
    fcT_p = _to_pds(np.asarray(fc_w, dtype=np.float32).T.astype(NPBF16), D)
    shared = {
        "wq_p": wq_p, "wk_p": wk_p, "wv_p": wv_p, "fcT_p": fcT_p,
        "ln_g": np.ascontiguousarray(np.asarray(ln_g).astype(NPBF16)),
        "ln_b": np.ascontiguousarray(np.asarray(ln_b).astype(NPBF16)),
    }
    in_maps = []
    for c in range(N_CORES):
        b, half = c // 2, c % 2
        sl = slice(half * SQ, (half + 1) * SQ)
        q_sl = q[b, sl, :]
        in_maps.append({
            "qT_sh": _to_pds((q_sl.T * TEMP_INV).astype(NPBF16), SQ),
            "kT_full": _to_pds(k[b].T.astype(NPBF16), S),
            "vT_full": _to_pds(v[b].T.astype(NPBF16), S),
            "mT_sh": _to_pds(mask[b, sl, :].T.astype(NPBF16), SQ),
            "qr_sh": np.ascontiguousarray(
                q_sl.astype(NPBF16).reshape(NQT, P, D).transpose(1, 0, 2)),
            **shared,
        })
    return in_maps


def run(inputs: dict, trace: bool = False):
    nc = _get_program()
    in_maps = make_in_maps(**inputs)
    res = run_bass_kernel_spmd(
        nc, in_maps, core_ids=list(range(N_CORES)), trace=trace)
    out = np.empty((B, S, D), dtype=np.float32)
    for c in range(N_CORES):
        b, half = c // 2, c % 2
        out[b, half * SQ:(half + 1) * SQ, :] = res.results[c]["out_sh"]
    return out, res


def kernel(q, k, v, mask, wq, wk, wv, fc_w, ln_g, ln_b):
    out, _ = run(dict(q=q, k=k, v=v, mask=mask, wq=wq, wk=wk, wv=wv,
                      fc_w=fc_w, ln_g=ln_g, ln_b=ln_b))
    return out
